# revision 20
# baseline (speedup 1.0000x reference)
"""Trainium2 Bass kernel for the CAM (channel attention module) problem.

Computation (per batch b):
    A = inputs[b] reshaped [N=4096, C=512]
    G = A^T A                       (channel Gram matrix, [C, C])
    attn = softmax(G, axis=-1)
    out[b] = gamma * (A @ attn^T) + A

Distribution: pure data-parallel over the batch dim: 16 batches over 8
NeuronCores = 2 batches/core. No collectives.

Design notes (v9):
  - Measured per-NC DMA ceiling is ~300 GB/s aggregate (queue count does
    not matter), so the 21MB of traffic (x bf16 8.4 + A^T fp8 4.2 +
    y bf16 8.4) is the wall.  The schedule is built to keep the DMA pipe
    full end-to-end: batch 0 runs its whole pipeline (gram -> softmax ->
    mm2 -> stores) WHILE batch 1's x / A^T loads stream in behind it,
    so b0's stores overlap b1's loads.  Per-ring FIFO emission order is
    arranged so every DMA is data-ready at its turn (no head-blocking).
  - fp8 gram operand A8 cast on-chip (DVE tensor_copy; saves 4.2MB of
    HBM vs a host-prepared fp8 copy).
  - gamma folded into the softmax weight row (W_rep = gamma/s_i), so po
    already holds gamma*(A attn^T) and the residual is a plain po + A:
      a-tiles: DVE tensor_tensor(po_psum + A)            (~690ns)
      q-tiles: extra PE matmul  po += I_bf16 @ A_tile    (~110ns PE)
               then a single ACT copy po -> out          (~830ns)
      u-tiles: ACT copy -> Sg, DVE bf16 add              (~440ns DVE)
  - Gs is bf16 (G ~ 4096 +- 300, softmax has ~3500 margin to underflow,
    so bf16 rounding is free); the exp-shift add runs at DVE 2x rate.
  - Partition-major HBM layouts (x[b,p,nt,c] etc) for fat descriptors.
"""

import sys

if "/opt/trn_rl_repo" not in sys.path:
    sys.path.insert(0, "/opt/trn_rl_repo")

import numpy as np

B, H, W, C = 16, 64, 64, 512
N = H * W                 # 4096
NCORES = 8
BPC = B // NCORES         # batches per core = 2
P = 128                   # partitions
NT = N // P               # 32 n-tiles
CT = C // P               # 4 channel tiles
NGRP = 4                  # n-tile groups per batch
GNT = NT // NGRP          # 8 n-tiles per group
OG = 4                    # n-tiles per output store group

_BUILD_CACHE = {}


def _ml_bf16():
    import ml_dtypes

    return np.dtype(ml_dtypes.bfloat16)


def build_bass(gamma_val: float):
    import concourse.bass as bass
    import concourse.bacc as bacc
    import concourse.tile as tile
    from concourse import mybir
    from contextlib import ExitStack

    f32 = mybir.dt.float32
    bf16 = mybir.dt.bfloat16
    f8 = mybir.dt.float8e4
    DR = mybir.MatmulPerfMode.DoubleRow
    Exp = mybir.ActivationFunctionType.Exp
    Alu = mybir.AluOpType
    AX = mybir.AxisListType

    nc = bacc.Bacc("TRN2", target_bir_lowering=False)
    x = nc.dram_tensor("x", [BPC, P, NT, C], bf16, kind="ExternalInput")
    xT8 = nc.dram_tensor("xT8", [BPC, P, CT, N], f8, kind="ExternalInput")
    ident = nc.dram_tensor("ident", [P, P], f32, kind="ExternalInput")
    ident_h = nc.dram_tensor("ident_h", [P, P], bf16, kind="ExternalInput")
    ones_f = nc.dram_tensor("ones_f", [1, P], f32, kind="ExternalInput")
    gamma_h = nc.dram_tensor("gamma_h", [1, P], bf16, kind="ExternalInput")
    y = nc.dram_tensor("y", [BPC, P, NT, C], bf16, kind="ExternalOutput")

    with tile.TileContext(nc) as tc, ExitStack() as ctx:
        singles = ctx.enter_context(tc.tile_pool(name="singles", bufs=1))
        pA = ctx.enter_context(tc.tile_pool(name="pA", bufs=2))
        pA8 = ctx.enter_context(tc.tile_pool(name="pA8", bufs=2))
        pAT = ctx.enter_context(tc.tile_pool(name="pAT", bufs=2))
        pGs = ctx.enter_context(tc.tile_pool(name="pGs", bufs=2))
        pSm = ctx.enter_context(tc.tile_pool(name="pSm", bufs=2))
        pTmp = ctx.enter_context(tc.tile_pool(name="pTmp", bufs=2))
        pTw = ctx.enter_context(tc.tile_pool(name="pTw", bufs=2))
        pSg = ctx.enter_context(tc.tile_pool(name="pSg", bufs=3))
        pOut = ctx.enter_context(tc.tile_pool(name="pOut", bufs=7))
        pG = ctx.enter_context(tc.tile_pool(name="pG", bufs=4, space="PSUM"))
        pPv = ctx.enter_context(tc.tile_pool(name="pPv", bufs=1, space="PSUM"))
        pPo = ctx.enter_context(tc.tile_pool(name="pPo", bufs=3, space="PSUM"))

        sb_ident = singles.tile([P, P], f32)
        nc.gpsimd.dma_start(out=sb_ident, in_=ident[:, :])
        sb_ident_h = singles.tile([P, P], bf16)
        nc.gpsimd.dma_start(out=sb_ident_h, in_=ident_h[:, :])
        sb_ones_f = singles.tile([1, P], f32)
        nc.gpsimd.dma_start(out=sb_ones_f, in_=ones_f[:, :])
        sb_gamma_h = singles.tile([1, P], bf16)
        nc.gpsimd.dma_start(out=sb_gamma_h, in_=gamma_h[:, :])

        st = [dict() for _ in range(BPC)]

        def emit_loads(b):
            """bf16 x load, groups alternating sync/gpsimd rings (two
            concurrent streams).  b0's first group in 2-nt chunks so the
            cast -> gram pipeline starts early."""
            Ab = pA.tile([P, NT, C], bf16, name=f"A_b{b}", tag="A")
            st[b]["A"] = Ab
            rings = [nc.sync, nc.gpsimd, nc.sync, nc.gpsimd]
            for g in range(NGRP):
                sl = slice(g * GNT, (g + 1) * GNT)
                if b == 0 and g == 0:
                    for h in range(GNT // 2):
                        nc.sync.dma_start(
                            out=Ab[:, 2 * h:2 * h + 2, :],
                            in_=x[b, :, 2 * h:2 * h + 2, :],
                        )
                else:
                    rings[g].dma_start(out=Ab[:, sl, :], in_=x[b, :, sl, :])

        def alloc_a8(b):
            st[b]["A8"] = pA8.tile([P, NT, C], f8, name=f"A8_b{b}", tag="A8")

        def emit_cast(b, fine_first=False):
            Ab, A8b = st[b]["A"], st[b]["A8"]
            chunks = []
            n0 = 0
            if fine_first:
                chunks += [(i * 2, (i + 1) * 2) for i in range(4)]   # g0 2-nt
                n0 = GNT
            while n0 < NT:
                chunks.append((n0, n0 + 4))
                n0 += 4
            for lo, hi in chunks:
                nc.vector.tensor_copy(
                    out=A8b[:, lo:hi, :], in_=Ab[:, lo:hi, :]
                )

        def alloc_at(b):
            st[b]["AT"] = pAT.tile([P, CT, N], f8, name=f"AT_b{b}", tag="AT")

        def emit_at(b, chunk_lo, chunk_hi):
            """A^T load in n-range chunks, alternating rings; mm2 can
            start on a partial A^T."""
            ATb = st[b]["AT"]
            NC4 = N // 4
            for c in range(chunk_lo, chunk_hi):
                sl = slice(c * NC4, (c + 1) * NC4)
                eng = nc.sync if c % 2 == 0 else nc.gpsimd
                eng.dma_start(out=ATb[:, :, sl], in_=xT8[b][:, :, sl])

        # Gram (upper-triangle blocks), fp8 DoubleRow
        def emit_gram(b):
            A8b = st[b]["A8"]
            G = [
                pG.tile([P, C], f32, name=f"G_b{b}c{ci}", tag="G")
                for ci in range(CT)
            ]
            NP2 = NT // 2
            for t in range(NP2):
                for ci in range(CT):
                    nc.tensor.matmul(
                        G[ci][:, ci * P:],
                        lhsT=A8b[:, 2 * t:2 * t + 2, ci * P:(ci + 1) * P],
                        rhs=A8b[:, 2 * t:2 * t + 2, ci * P:],
                        start=(t == 0),
                        stop=(t == NP2 - 1),
                        perf_mode=DR,
                    )
            st[b]["G"] = G

        # G rows PSUM->SBUF as bf16 (frees the G banks) + row-max (negated)
        def emit_stats(b):
            G = st[b]["G"]
            Gs = pGs.tile([P, CT, C], bf16, name=f"Gs_b{b}", tag="Gs")
            for ci in range(CT):
                eng = nc.vector.tensor_copy if ci % 2 == 0 else nc.scalar.copy
                eng(out=Gs[:, ci, ci * P:], in_=G[ci][:, ci * P:])
            negm = pSm.tile([P, CT], f32, name=f"negm_b{b}", tag="negm")
            for it in range(CT):
                nc.vector.tensor_reduce(
                    out=negm[:, it:it + 1],
                    in_=Gs[:, it, it * P:],
                    axis=AX.X,
                    op=Alu.max,
                    negate=True,
                )
            st[b]["Gs"] = Gs
            st[b]["negm"] = negm

        # softmax tail, emitted standalone (runs between gram and mm2)
        def emit_softmax(b):
            ve = nc.vector
            Gs = st[b]["Gs"]
            negm = st[b]["negm"]
            s_acc = pSm.tile([P, CT], f32, name=f"s_b{b}", tag="s")
            wrec = pSm.tile([P, CT], f32, name=f"w_b{b}", tag="w")
            Tw8 = pTw.tile([P, CT, C], f8, name=f"Tw8_b{b}", tag="Tw")
            st[b]["Tw"] = Tw8

            # rebuild lower-triangle blocks of Gs by PE transposes
            QUADS = [[(1, 0), (2, 0), (2, 1), (3, 0)], [(3, 1), (3, 2)]]
            for qi, quad in enumerate(QUADS):
                trq = pPv.tile([P, C], bf16, name=f"trq_b{b}_{qi}", tag="pv")
                for q, (it, jt) in enumerate(quad):
                    nc.tensor.transpose(
                        out=trq[:, q * P:(q + 1) * P],
                        in_=Gs[:, jt, it * P:(it + 1) * P],
                        identity=sb_ident_h,
                    )
                for q, (it, jt) in enumerate(quad):
                    nc.scalar.copy(out=Gs[:, it, jt * P:(jt + 1) * P],
                                   in_=trq[:, q * P:(q + 1) * P])

            for it in range(CT):
                S = pTmp.tile([P, C], bf16, name=f"S_b{b}t{it}", tag="S")
                nc.scalar.activation(
                    out=S,
                    in_=Gs[:, it, :],
                    func=Exp,
                    bias=negm[:, it:it + 1],
                    scale=1.0,
                    accum_out=s_acc[:, it:it + 1],
                )
            nc.vector.reciprocal(out=wrec, in_=s_acc)

            def col_to_row(src, row):
                vps = pPv.tile([1, C], f32, name=f"vps_{id(row)}", tag="pv")
                for it in range(CT):
                    nc.tensor.transpose(
                        out=vps[0:1, it * P:(it + 1) * P],
                        in_=src[:, it:it + 1],
                        identity=sb_ident,
                    )
                nc.scalar.copy(out=row, in_=vps)

            negm_row = pSm.tile([1, C], f32, name=f"negmrow_b{b}", tag="nrow")
            col_to_row(negm, negm_row)
            w_row = pSm.tile([1, C], bf16, name=f"wrow_b{b}", tag="wrow")
            col_to_row(wrec, w_row)

            NegM_rep = pSm.tile([P, C], bf16, name=f"negmrep_b{b}", tag="mrep")
            W_rep = pSm.tile([P, C], bf16, name=f"wrep_b{b}", tag="wrep")

            def rank1(onesv, row, rep):
                ps = pPv.tile([P, C], f32, name=f"rep_{id(rep)}", tag="pv")
                nc.tensor.matmul(ps, lhsT=onesv, rhs=row, start=True, stop=True)
                nc.scalar.copy(out=rep, in_=ps)

            rank1(sb_ones_f, negm_row, NegM_rep)
            # gamma folded here: W_rep[p, i] = gamma * w_i
            rank1(sb_gamma_h, w_row, W_rep)

            # T_w[j, i] = gamma * exp(G[j, i] - m_i) * w_i   (G symmetric)
            for jt in range(CT):
                tmp = pTmp.tile([P, C], bf16, name=f"tmp_b{b}j{jt}", tag="tmp")
                ve.tensor_tensor(
                    out=tmp, in0=Gs[:, jt, :], in1=NegM_rep, op=Alu.add
                )
                Texp = pTmp.tile([P, C], bf16, name=f"Texp_b{b}j{jt}",
                                 tag="Texp")
                nc.scalar.activation(out=Texp, in_=tmp, func=Exp)
                ve.tensor_mul(out=Tw8[:, jt, :], in0=Texp, in1=W_rep)

        # second matmul + residual (po + A, gamma already in Tw) + store
        def emit_mm2(b):
            Ab = st[b]["A"]
            ATb = st[b]["AT"]
            Tw8 = st[b]["Tw"]
            NOG = NT // OG
            for og in range(NOG):
                outg = pOut.tile(
                    [P, OG, C], bf16, name=f"out_b{b}g{og}", tag="out"
                )
                for k in range(OG):
                    nt = og * OG + k
                    # tile kind: a=DVE-TT, q=PE-ident+ACT, u=ACT+DVE-add.
                    # b0's era overlaps cast_b1 on DVE -> more q;
                    # b1's era has free DVE -> more a.
                    if b == 0:
                        kind = ("a", "q",
                                "a" if og % 2 == 0 else "q", "q")[k]
                    else:
                        kind = ("a", "q", "a",
                                "u" if og % 2 == 0 else "q")[k]
                    po = pPo.tile([P, C], f32, name=f"po_b{b}n{nt}", tag="po")
                    for u in range(CT // 2):
                        nc.tensor.matmul(
                            po,
                            lhsT=ATb[:, 2 * u:2 * u + 2, nt * P:(nt + 1) * P],
                            rhs=Tw8[:, 2 * u:2 * u + 2, :],
                            start=(u == 0),
                            stop=(u == CT // 2 - 1 and kind != "q"),
                            perf_mode=DR,
                        )
                    if kind == "q":
                        # po += I @ A_tile on the PE; drain is 1 ACT copy
                        nc.tensor.matmul(
                            po,
                            lhsT=sb_ident_h,
                            rhs=Ab[:, nt, :],
                            start=False,
                            stop=True,
                        )
                        nc.scalar.copy(out=outg[:, k, :], in_=po)
                    elif kind == "a":
                        nc.vector.tensor_tensor(
                            out=outg[:, k, :], in0=po, in1=Ab[:, nt, :],
                            op=Alu.add,
                        )
                    else:
                        Sgt = pSg.tile([P, C], bf16,
                                       name=f"Sg_b{b}n{nt}", tag="Sg")
                        nc.scalar.copy(out=Sgt, in_=po)
                        nc.vector.tensor_tensor(
                            out=outg[:, k, :], in0=Sgt, in1=Ab[:, nt, :],
                            op=Alu.add,
                        )
                dst = y[b, :, og * OG:(og + 1) * OG, :]
                if b == 1 and og == NOG - 1:
                    # split the final store across both rings (shorter tail)
                    half = OG // 2
                    nc.gpsimd.dma_start(out=dst[:, :half, :],
                                        in_=outg[:, :half, :])
                    nc.sync.dma_start(out=dst[:, half:, :],
                                      in_=outg[:, half:, :])
                else:
                    eng = nc.sync if og % 2 == 0 else nc.gpsimd
                    eng.dma_start(out=dst, in_=outg)

        # ---- PE warm-up: keep HAM busy before the first loads land -------
        warm_sb = pSm.tile([P, P], bf16, name="warm_sb", tag="warmsb")
        nc.vector.memset(warm_sb, 0.0)
        warm_ps = pPo.tile([P, P], f32, name="warm_ps", tag="po")
        for _ in range(40):
            nc.tensor.matmul(warm_ps, lhsT=warm_sb, rhs=warm_sb,
                             start=True, stop=True)

        # ---- schedule: b0's full pipeline overlaps b1's loads ------------
        alloc_a8(0)
        alloc_a8(1)
        alloc_at(0)
        alloc_at(1)
        emit_loads(0)                  # x_b0 dual-ring
        emit_at(0, 0, 4)               # A^T_b0, 4 chunks across rings
        emit_loads(1)                  # x_b1 streams behind b0's compute
        emit_at(1, 0, 2)               # A^T_b1 head chunks
        emit_cast(0, fine_first=True)
        emit_gram(0)
        emit_stats(0)
        emit_softmax(0)
        emit_mm2(0)                    # b0 stores interleave with b1 loads
        emit_at(1, 2, 4)               # A^T_b1 tail chunks (after b0 stores)
        emit_cast(1)
        emit_gram(1)
        emit_stats(1)
        emit_softmax(1)
        emit_mm2(1)

    nc.compile()
    return nc


def run(inputs_arr: np.ndarray, gamma_val: float, trace: bool = False):
    """Compile + run on the 8 cores. Returns (output [16,64,64,512], results)."""
    from concourse.bass_utils import run_bass_kernel_spmd

    key = round(float(gamma_val), 12)
    if key not in _BUILD_CACHE:
        _BUILD_CACHE[key] = build_bass(float(gamma_val))
    nc = _BUILD_CACHE[key]

    import ml_dtypes

    bf16 = _ml_bf16()
    f8 = np.dtype(ml_dtypes.float8_e4m3)
    xs = np.asarray(inputs_arr, dtype=np.float32).reshape(B, N, C).astype(bf16)
    # partition-major tiles: x[b, p, nt, c]
    xs_t = np.ascontiguousarray(
        xs.reshape(B, NT, P, C).transpose(0, 2, 1, 3)
    )
    # xT8[b, p, jt, n]
    xsT8 = (
        xs.astype(f8).transpose(0, 2, 1)
        .reshape(B, CT, P, N).transpose(0, 2, 1, 3)
    )
    xsT8 = np.ascontiguousarray(xsT8)
    eye = np.eye(P, dtype=np.float32)
    eye_h = eye.astype(bf16)
    ones_f = np.ones((1, P), dtype=np.float32)
    gamma_h = np.full((1, P), gamma_val, dtype=np.float32).astype(bf16)
    in_maps = [
        {
            "x": xs_t[c * BPC:(c + 1) * BPC],
            "xT8": xsT8[c * BPC:(c + 1) * BPC],
            "ident": eye,
            "ident_h": eye_h,
            "ones_f": ones_f,
            "gamma_h": gamma_h,
        }
        for c in range(NCORES)
    ]
    res = run_bass_kernel_spmd(nc, in_maps, list(range(NCORES)), trace=trace)
    out = np.concatenate(
        [np.asarray(res.results[c]["y"]) for c in range(NCORES)], axis=0
    )
    out = out.transpose(0, 2, 1, 3).reshape(B, N, C)
    return out.astype(np.float32).reshape(B, H, W, C), res


def kernel(inputs: np.ndarray, gamma: np.ndarray) -> np.ndarray:
    gamma_val = float(np.asarray(gamma).reshape(-1)[0])
    out, _ = run(inputs, gamma_val, trace=False)
    return out.astype(np.float32)


if __name__ == "__main__":
    rng = np.random.default_rng(0)
    inp = rng.standard_normal((B, H, W, C), dtype=np.float32)
    gam = np.zeros((1,), dtype=np.float32)
    out = kernel(inp, gam)
    print("shape", out.shape, "dtype", out.dtype)
    print("max|out - inp| =", np.abs(out - inp).max())


# revision 25
# speedup vs baseline: 1.0750x; 1.0750x over previous
"""Trainium2 Bass kernel for the CAM (channel attention module) problem.

Computation (per batch b):
    A = inputs[b] reshaped [N=4096, C=512]
    G = A^T A                       (channel Gram matrix, [C, C])
    attn = softmax(G, axis=-1)
    out[b] = gamma * (A @ attn^T) + A

Distribution: pure data-parallel over the batch dim: 16 batches over 8
NeuronCores = 2 batches/core. No collectives.

Design notes (v9):
  - Measured per-NC DMA ceiling is ~300 GB/s aggregate (queue count does
    not matter), so the 21MB of traffic (x bf16 8.4 + A^T fp8 4.2 +
    y bf16 8.4) is the wall.  The schedule is built to keep the DMA pipe
    full end-to-end: batch 0 runs its whole pipeline (gram -> softmax ->
    mm2 -> stores) WHILE batch 1's x / A^T loads stream in behind it,
    so b0's stores overlap b1's loads.  Per-ring FIFO emission order is
    arranged so every DMA is data-ready at its turn (no head-blocking).
  - fp8 gram operand A8 cast on-chip (DVE tensor_copy; saves 4.2MB of
    HBM vs a host-prepared fp8 copy).
  - gamma folded into the softmax weight row (W_rep = gamma/s_i), so po
    already holds gamma*(A attn^T) and the residual is a plain po + A:
      a-tiles: DVE tensor_tensor(po_psum + A)            (~690ns)
      q-tiles: extra PE matmul  po += I_bf16 @ A_tile    (~110ns PE)
               then a single ACT copy po -> out          (~830ns)
      u-tiles: ACT copy -> Sg, DVE bf16 add              (~440ns DVE)
  - Gs is bf16 (G ~ 4096 +- 300, softmax has ~3500 margin to underflow,
    so bf16 rounding is free); the exp-shift add runs at DVE 2x rate.
  - Partition-major HBM layouts (x[b,p,nt,c] etc) for fat descriptors.
"""

import sys

if "/opt/trn_rl_repo" not in sys.path:
    sys.path.insert(0, "/opt/trn_rl_repo")

import numpy as np

B, H, W, C = 16, 64, 64, 512
N = H * W                 # 4096
NCORES = 8
BPC = B // NCORES         # batches per core = 2
P = 128                   # partitions
NT = N // P               # 32 n-tiles
CT = C // P               # 4 channel tiles
NGRP = 4                  # n-tile groups per batch
GNT = NT // NGRP          # 8 n-tiles per group
OG = 4                    # n-tiles per output store group

_BUILD_CACHE = {}


def _ml_bf16():
    import ml_dtypes

    return np.dtype(ml_dtypes.bfloat16)


def build_bass(gamma_val: float):
    import concourse.bass as bass
    import concourse.bacc as bacc
    import concourse.tile as tile
    from concourse import mybir
    from contextlib import ExitStack

    f32 = mybir.dt.float32
    bf16 = mybir.dt.bfloat16
    f8 = mybir.dt.float8e4
    DR = mybir.MatmulPerfMode.DoubleRow
    Exp = mybir.ActivationFunctionType.Exp
    Alu = mybir.AluOpType
    AX = mybir.AxisListType

    nc = bacc.Bacc("TRN2", target_bir_lowering=False)
    x = nc.dram_tensor("x", [BPC, P, NT, C], bf16, kind="ExternalInput")
    xT8 = nc.dram_tensor("xT8", [BPC, P, CT, N], f8, kind="ExternalInput")
    ident = nc.dram_tensor("ident", [P, P], f32, kind="ExternalInput")
    ident_h = nc.dram_tensor("ident_h", [P, P], bf16, kind="ExternalInput")
    ones_f = nc.dram_tensor("ones_f", [1, P], f32, kind="ExternalInput")
    gamma_h = nc.dram_tensor("gamma_h", [1, P], bf16, kind="ExternalInput")
    y = nc.dram_tensor("y", [BPC, P, NT, C], bf16, kind="ExternalOutput")

    with tile.TileContext(nc) as tc, ExitStack() as ctx:
        singles = ctx.enter_context(tc.tile_pool(name="singles", bufs=1))
        pA = ctx.enter_context(tc.tile_pool(name="pA", bufs=2))
        pA8 = ctx.enter_context(tc.tile_pool(name="pA8", bufs=2))
        pAT = ctx.enter_context(tc.tile_pool(name="pAT", bufs=2))
        pGs = ctx.enter_context(tc.tile_pool(name="pGs", bufs=2))
        pSm = ctx.enter_context(tc.tile_pool(name="pSm", bufs=2))
        pTmp = ctx.enter_context(tc.tile_pool(name="pTmp", bufs=2))
        pTw = ctx.enter_context(tc.tile_pool(name="pTw", bufs=2))
        pSg = ctx.enter_context(tc.tile_pool(name="pSg", bufs=3))
        pOut = ctx.enter_context(tc.tile_pool(name="pOut", bufs=7))
        pG = ctx.enter_context(tc.tile_pool(name="pG", bufs=4, space="PSUM"))
        pPv = ctx.enter_context(tc.tile_pool(name="pPv", bufs=1, space="PSUM"))
        pPo = ctx.enter_context(tc.tile_pool(name="pPo", bufs=3, space="PSUM"))

        sb_ident = singles.tile([P, P], f32)
        nc.gpsimd.dma_start(out=sb_ident, in_=ident[:, :])
        sb_ident_h = singles.tile([P, P], bf16)
        nc.gpsimd.dma_start(out=sb_ident_h, in_=ident_h[:, :])
        sb_ones_f = singles.tile([1, P], f32)
        nc.gpsimd.dma_start(out=sb_ones_f, in_=ones_f[:, :])
        sb_gamma_h = singles.tile([1, P], bf16)
        nc.gpsimd.dma_start(out=sb_gamma_h, in_=gamma_h[:, :])

        st = [dict() for _ in range(BPC)]

        def emit_loads(b):
            """bf16 x load, groups alternating sync/gpsimd rings (two
            concurrent streams).  b0's first group in 2-nt chunks so the
            cast -> gram pipeline starts early."""
            Ab = pA.tile([P, NT, C], bf16, name=f"A_b{b}", tag="A")
            st[b]["A"] = Ab
            # ALL loads on the sync (HWDGE) ring: it executes FIFO, so
            # arrival order matches consumption order.  (SWDGE round-robins
            # its whole queue concurrently -- everything lands late.)
            for g in range(NGRP):
                sl = slice(g * GNT, (g + 1) * GNT)
                if b == 0 and g == 0:
                    for h in range(GNT // 2):
                        nc.sync.dma_start(
                            out=Ab[:, 2 * h:2 * h + 2, :],
                            in_=x[b, :, 2 * h:2 * h + 2, :],
                        )
                else:
                    nc.sync.dma_start(out=Ab[:, sl, :], in_=x[b, :, sl, :])

        def alloc_a8(b):
            st[b]["A8"] = pA8.tile([P, NT, C], f8, name=f"A8_b{b}", tag="A8")

        def emit_cast(b, fine_first=False):
            Ab, A8b = st[b]["A"], st[b]["A8"]
            chunks = []
            n0 = 0
            if fine_first:
                chunks += [(i * 2, (i + 1) * 2) for i in range(4)]   # g0 2-nt
                n0 = GNT
            while n0 < NT:
                chunks.append((n0, n0 + 4))
                n0 += 4
            for lo, hi in chunks:
                nc.vector.tensor_copy(
                    out=A8b[:, lo:hi, :], in_=Ab[:, lo:hi, :]
                )

        def alloc_at(b):
            st[b]["AT"] = pAT.tile([P, CT, N], f8, name=f"AT_b{b}", tag="AT")

        def emit_at(b, chunk_lo, chunk_hi):
            """A^T load in n-range chunks (sync ring); mm2 can start on a
            partial A^T."""
            ATb = st[b]["AT"]
            NC4 = N // 4
            for c in range(chunk_lo, chunk_hi):
                sl = slice(c * NC4, (c + 1) * NC4)
                nc.sync.dma_start(out=ATb[:, :, sl], in_=xT8[b][:, :, sl])

        # Gram (upper-triangle blocks), fp8 DoubleRow
        def emit_gram(b):
            A8b = st[b]["A8"]
            G = [
                pG.tile([P, C], f32, name=f"G_b{b}c{ci}", tag="G")
                for ci in range(CT)
            ]
            NP2 = NT // 2
            for t in range(NP2):
                for ci in range(CT):
                    nc.tensor.matmul(
                        G[ci][:, ci * P:],
                        lhsT=A8b[:, 2 * t:2 * t + 2, ci * P:(ci + 1) * P],
                        rhs=A8b[:, 2 * t:2 * t + 2, ci * P:],
                        start=(t == 0),
                        stop=(t == NP2 - 1),
                        perf_mode=DR,
                    )
            st[b]["G"] = G

        # G rows PSUM->SBUF as bf16 (frees the G banks) + row-max (negated)
        def emit_stats(b):
            G = st[b]["G"]
            Gs = pGs.tile([P, CT, C], bf16, name=f"Gs_b{b}", tag="Gs")
            for ci in range(CT):
                eng = nc.vector.tensor_copy if ci % 2 == 0 else nc.scalar.copy
                eng(out=Gs[:, ci, ci * P:], in_=G[ci][:, ci * P:])
            negm = pSm.tile([P, CT], f32, name=f"negm_b{b}", tag="negm")
            for it in range(CT):
                nc.vector.tensor_reduce(
                    out=negm[:, it:it + 1],
                    in_=Gs[:, it, it * P:],
                    axis=AX.X,
                    op=Alu.max,
                    negate=True,
                )
            st[b]["Gs"] = Gs
            st[b]["negm"] = negm

        # softmax tail, emitted standalone (runs between gram and mm2)
        def emit_softmax(b):
            ve = nc.vector
            Gs = st[b]["Gs"]
            negm = st[b]["negm"]
            s_acc = pSm.tile([P, CT], f32, name=f"s_b{b}", tag="s")
            wrec = pSm.tile([P, CT], f32, name=f"w_b{b}", tag="w")
            Tw8 = pTw.tile([P, CT, C], f8, name=f"Tw8_b{b}", tag="Tw")
            st[b]["Tw"] = Tw8

            # rebuild lower-triangle blocks of Gs by PE transposes
            QUADS = [[(1, 0), (2, 0), (2, 1), (3, 0)], [(3, 1), (3, 2)]]
            for qi, quad in enumerate(QUADS):
                trq = pPv.tile([P, C], bf16, name=f"trq_b{b}_{qi}", tag="pv")
                for q, (it, jt) in enumerate(quad):
                    nc.tensor.transpose(
                        out=trq[:, q * P:(q + 1) * P],
                        in_=Gs[:, jt, it * P:(it + 1) * P],
                        identity=sb_ident_h,
                    )
                for q, (it, jt) in enumerate(quad):
                    nc.scalar.copy(out=Gs[:, it, jt * P:(jt + 1) * P],
                                   in_=trq[:, q * P:(q + 1) * P])

            for it in range(CT):
                S = pTmp.tile([P, C], bf16, name=f"S_b{b}t{it}", tag="S")
                nc.scalar.activation(
                    out=S,
                    in_=Gs[:, it, :],
                    func=Exp,
                    bias=negm[:, it:it + 1],
                    scale=1.0,
                    accum_out=s_acc[:, it:it + 1],
                )
            nc.vector.reciprocal(out=wrec, in_=s_acc)

            def col_to_row(src, row):
                vps = pPv.tile([1, C], f32, name=f"vps_{id(row)}", tag="pv")
                for it in range(CT):
                    nc.tensor.transpose(
                        out=vps[0:1, it * P:(it + 1) * P],
                        in_=src[:, it:it + 1],
                        identity=sb_ident,
                    )
                nc.scalar.copy(out=row, in_=vps)

            negm_row = pSm.tile([1, C], f32, name=f"negmrow_b{b}", tag="nrow")
            col_to_row(negm, negm_row)
            w_row = pSm.tile([1, C], bf16, name=f"wrow_b{b}", tag="wrow")
            col_to_row(wrec, w_row)

            NegM_rep = pSm.tile([P, C], bf16, name=f"negmrep_b{b}", tag="mrep")
            W_rep = pSm.tile([P, C], bf16, name=f"wrep_b{b}", tag="wrep")

            def rank1(onesv, row, rep):
                ps = pPv.tile([P, C], f32, name=f"rep_{id(rep)}", tag="pv")
                nc.tensor.matmul(ps, lhsT=onesv, rhs=row, start=True, stop=True)
                nc.scalar.copy(out=rep, in_=ps)

            rank1(sb_ones_f, negm_row, NegM_rep)
            # gamma folded here: W_rep[p, i] = gamma * w_i
            rank1(sb_gamma_h, w_row, W_rep)

            # T_w[j, i] = gamma * exp(G[j, i] - m_i) * w_i   (G symmetric)
            for jt in range(CT):
                tmp = pTmp.tile([P, C], bf16, name=f"tmp_b{b}j{jt}", tag="tmp")
                ve.tensor_tensor(
                    out=tmp, in0=Gs[:, jt, :], in1=NegM_rep, op=Alu.add
                )
                Texp = pTmp.tile([P, C], bf16, name=f"Texp_b{b}j{jt}",
                                 tag="Texp")
                nc.scalar.activation(out=Texp, in_=tmp, func=Exp)
                ve.tensor_mul(out=Tw8[:, jt, :], in0=Texp, in1=W_rep)

        # second matmul + residual (po + A, gamma already in Tw) + store
        def emit_mm2(b, side_ops=()):
            side = list(side_ops)
            Ab = st[b]["A"]
            ATb = st[b]["AT"]
            Tw8 = st[b]["Tw"]
            NOG = NT // OG
            for og in range(NOG):
                outg = pOut.tile(
                    [P, OG, C], bf16, name=f"out_b{b}g{og}", tag="out"
                )
                for k in range(OG):
                    nt = og * OG + k
                    # tile kind: a=DVE-TT, q=PE-ident+ACT, u=ACT+DVE-add.
                    # b0's era overlaps cast_b1 on DVE -> more q;
                    # b1's era has free DVE -> more a.
                    if b == 0:
                        kind = ("a", "q",
                                "a" if og % 2 == 0 else "q", "q")[k]
                    else:
                        kind = ("a", "q", "a",
                                "u" if og % 2 == 0 else "q")[k]
                    po = pPo.tile([P, C], f32, name=f"po_b{b}n{nt}", tag="po")
                    for u in range(CT // 2):
                        nc.tensor.matmul(
                            po,
                            lhsT=ATb[:, 2 * u:2 * u + 2, nt * P:(nt + 1) * P],
                            rhs=Tw8[:, 2 * u:2 * u + 2, :],
                            start=(u == 0),
                            stop=(u == CT // 2 - 1 and kind != "q"),
                            perf_mode=DR,
                        )
                    if kind == "q":
                        # po += I @ A_tile on the PE; drain is 1 ACT copy
                        nc.tensor.matmul(
                            po,
                            lhsT=sb_ident_h,
                            rhs=Ab[:, nt, :],
                            start=False,
                            stop=True,
                        )
                        nc.scalar.copy(out=outg[:, k, :], in_=po)
                    elif kind == "a":
                        nc.vector.tensor_tensor(
                            out=outg[:, k, :], in0=po, in1=Ab[:, nt, :],
                            op=Alu.add,
                        )
                    else:
                        Sgt = pSg.tile([P, C], bf16,
                                       name=f"Sg_b{b}n{nt}", tag="Sg")
                        nc.scalar.copy(out=Sgt, in_=po)
                        nc.vector.tensor_tensor(
                            out=outg[:, k, :], in0=Sgt, in1=Ab[:, nt, :],
                            op=Alu.add,
                        )
                    if side and k == 3:
                        side.pop(0)()
                dst = y[b, :, og * OG:(og + 1) * OG, :]
                if b == 1 and og == NOG - 1:
                    # split the final store across both rings (shorter tail)
                    half = OG // 2
                    nc.gpsimd.dma_start(out=dst[:, :half, :],
                                        in_=outg[:, :half, :])
                    nc.sync.dma_start(out=dst[:, half:, :],
                                      in_=outg[:, half:, :])
                else:
                    nc.gpsimd.dma_start(out=dst, in_=outg)
            while side:
                side.pop(0)()

        # ---- PE warm-up: keep HAM busy before the first loads land -------
        warm_sb = pSm.tile([P, P], bf16, name="warm_sb", tag="warmsb")
        nc.vector.memset(warm_sb, 0.0)
        warm_ps = pPo.tile([P, P], f32, name="warm_ps", tag="po")
        for _ in range(40):
            nc.tensor.matmul(warm_ps, lhsT=warm_sb, rhs=warm_sb,
                             start=True, stop=True)

        # ---- schedule: b0's full pipeline overlaps b1's loads ------------
        alloc_a8(0)
        alloc_a8(1)
        alloc_at(0)
        alloc_at(1)
        emit_loads(0)                  # sync FIFO: x_b0 ...
        emit_at(0, 0, 4)               # ... then A^T_b0 ...
        emit_loads(1)                  # ... then x_b1 (lands during mm2_b0)
        emit_at(1, 0, 4)               # ... then A^T_b1
        emit_cast(0, fine_first=True)
        emit_gram(0)
        emit_stats(0)
        emit_softmax(0)
        # cast_b1 chunks ride in mm2_b0's slot stream (DVE has slack there,
        # and x_b1 groups land just ahead of each slot's turn)
        cast_slots = [
            (lambda lo=lo: nc.vector.tensor_copy(
                out=st[1]["A8"][:, lo:lo + 4, :],
                in_=st[1]["A"][:, lo:lo + 4, :]))
            for lo in range(0, NT, 4)
        ]
        emit_mm2(0, side_ops=cast_slots)   # b0 stores overlap b1 loads
        emit_gram(1)
        emit_stats(1)
        emit_softmax(1)
        emit_mm2(1)

    nc.compile()
    return nc


def run(inputs_arr: np.ndarray, gamma_val: float, trace: bool = False):
    """Compile + run on the 8 cores. Returns (output [16,64,64,512], results)."""
    from concourse.bass_utils import run_bass_kernel_spmd

    key = round(float(gamma_val), 12)
    if key not in _BUILD_CACHE:
        _BUILD_CACHE[key] = build_bass(float(gamma_val))
    nc = _BUILD_CACHE[key]

    import ml_dtypes

    bf16 = _ml_bf16()
    f8 = np.dtype(ml_dtypes.float8_e4m3)
    xs = np.asarray(inputs_arr, dtype=np.float32).reshape(B, N, C).astype(bf16)
    # partition-major tiles: x[b, p, nt, c]
    xs_t = np.ascontiguousarray(
        xs.reshape(B, NT, P, C).transpose(0, 2, 1, 3)
    )
    # xT8[b, p, jt, n]
    xsT8 = (
        xs.astype(f8).transpose(0, 2, 1)
        .reshape(B, CT, P, N).transpose(0, 2, 1, 3)
    )
    xsT8 = np.ascontiguousarray(xsT8)
    eye = np.eye(P, dtype=np.float32)
    eye_h = eye.astype(bf16)
    ones_f = np.ones((1, P), dtype=np.float32)
    gamma_h = np.full((1, P), gamma_val, dtype=np.float32).astype(bf16)
    in_maps = [
        {
            "x": xs_t[c * BPC:(c + 1) * BPC],
            "xT8": xsT8[c * BPC:(c + 1) * BPC],
            "ident": eye,
            "ident_h": eye_h,
            "ones_f": ones_f,
            "gamma_h": gamma_h,
        }
        for c in range(NCORES)
    ]
    res = run_bass_kernel_spmd(nc, in_maps, list(range(NCORES)), trace=trace)
    out = np.concatenate(
        [np.asarray(res.results[c]["y"]) for c in range(NCORES)], axis=0
    )
    out = out.transpose(0, 2, 1, 3).reshape(B, N, C)
    return out.astype(np.float32).reshape(B, H, W, C), res


def kernel(inputs: np.ndarray, gamma: np.ndarray) -> np.ndarray:
    gamma_val = float(np.asarray(gamma).reshape(-1)[0])
    out, _ = run(inputs, gamma_val, trace=False)
    return out.astype(np.float32)


if __name__ == "__main__":
    rng = np.random.default_rng(0)
    inp = rng.standard_normal((B, H, W, C), dtype=np.float32)
    gam = np.zeros((1,), dtype=np.float32)
    out = kernel(inp, gam)
    print("shape", out.shape, "dtype", out.dtype)
    print("max|out - inp| =", np.abs(out - inp).max())
